# revision 19
# baseline (speedup 1.0000x reference)
"""GCN (3x GCNConv + global max pool + MLP) on 8 Trainium2 NeuronCores.

Strategy (data-parallel over graphs, per sharding hint):
 - Nodes laid out graph-padded: each graph gets a fixed slot of L_PAD columns;
   core c owns graphs [8c, 8c+8) -> M_PAD = 8*L_PAD padded node slots.
 - Per layer: p = h @ W computed for own nodes (fp16), AllGather -> replicated
   fp16 p table with rows padded to 384 elems (768B, dma_gather needs 256B
   multiples), per-edge gather of p[src] rows via dma_gather batched per
   (superblock of 4 dst blocks, src half), aggregation as PE matmuls with
   one-hot selection matrices S built ON-CHIP per 128-msg tile:
       S[p, dl] = (iota[dl] == dl_col[p]) * w_col[p]
   via a single tensor_scalar (alternating DVE / Activation engines).
   Self-loops are ordinary messages (w = dinv^2). Bias enters as a rank-1
   mask x brow matmul so padded slots stay exactly 0; relu on evacuation;
   transpose to feature-major for the next layer's lhsT.
 - Pooling: per-graph column-slice reduce_max (pads are exactly 0, relu
   output >= 0, so padding never changes the max). Pooled vectors
   AllGathered, MLP head computed redundantly on every core (fp32).
"""
import os
import sys
import numpy as np

for _p in ('/opt/trn_rl_repo', '/root/.axon_site/_ro/trn_rl_repo'):
    if os.path.isdir(_p) and _p not in sys.path:
        sys.path.insert(0, _p)

N_CORES = 8
N_NODES = 50000
D = 320
DP = 512          # table row padded to 512 fp8 elems = 512 B (256B multiple)
N_GRAPHS = 64
GPC = N_GRAPHS // N_CORES  # graphs per core
SB = 4                     # dst blocks per gather superblock


def _preprocess(x, edge_index, batch):
    """Build per-core gather indices, S-build tables and layouts."""
    batch = np.asarray(batch).astype(np.int64)
    src = np.asarray(edge_index[0]).astype(np.int64)
    dst = np.asarray(edge_index[1]).astype(np.int64)
    counts = np.bincount(batch, minlength=N_GRAPHS)
    L_PAD = max(896, int(-(-counts.max() // 128)) * 128)
    M_PAD = GPC * L_PAD
    TOT = N_CORES * M_PAD
    HALF = TOT // 2
    assert HALF <= 32767, (L_PAD, HALF)
    NB = M_PAD // 128   # dst blocks per core
    NSB = -(-NB // SB)  # superblocks per core

    # graph -> (core, slot) permutation: sort graphs by in-edge load so
    # similarly-sized graphs share a slot index across cores, keeping the
    # uniform per-(block,half) tile count K_FIX tight.
    indeg_n = np.bincount(dst, minlength=N_NODES) + 1
    gload = np.zeros(N_GRAPHS, np.int64)
    np.add.at(gload, batch, indeg_n)
    gorder = np.argsort(gload, kind='stable')        # graph ids, light -> heavy
    # slot s of core c holds graph gorder[s * N_CORES + c]
    slotcore_of_graph = np.empty(N_GRAPHS, np.int64)
    slotcore_of_graph[gorder] = np.arange(N_GRAPHS)  # s*N_CORES + c
    g_slot = slotcore_of_graph // N_CORES            # slot index per graph
    g_core = slotcore_of_graph % N_CORES             # core per graph
    perm_out = gorder.reshape(GPC, N_CORES).T.reshape(-1)  # out col (c*GPC+s) -> graph

    # within-graph relabeling: greedy two-half balance into L_PAD/128 blocks
    # (a message's half is fixed by its source graph's core group).
    NBLK = L_PAD // 128
    MH = M_PAD // 2
    own_half = (g_slot[batch] >= (GPC // 2)).astype(np.int64)
    src_half = own_half[src]
    d0 = np.bincount(dst[src_half == 0], minlength=N_NODES)
    d1 = np.bincount(dst[src_half == 1], minlength=N_NODES)
    d0 = d0 + (own_half == 0)
    d1 = d1 + (own_half == 1)
    pos_in_graph = np.empty(N_NODES, np.int64)
    for gi in range(N_GRAPHS):
        nodes = np.where(batch == gi)[0]
        dd0, dd1 = d0[nodes], d1[nodes]
        order_g = np.argsort(-(dd0 + dd1), kind='stable')
        load = np.zeros((NBLK, 2), np.int64)
        fill = np.zeros(NBLK, np.int64)
        for i in order_g:
            n0, n1 = dd0[i], dd1[i]
            best, bestv = -1, None
            for b in range(NBLK):
                if fill[b] >= 128:
                    continue
                v = (load[b, 0] + n0) ** 2 + (load[b, 1] + n1) ** 2
                if bestv is None or v < bestv:
                    bestv, best = v, b
            load[best, 0] += n0
            load[best, 1] += n1
            pos_in_graph[nodes[i]] = best * 128 + fill[best]
            fill[best] += 1
    pos = g_core[batch] * M_PAD + g_slot[batch] * L_PAD + pos_in_graph

    deg = np.bincount(dst, minlength=N_NODES).astype(np.float64) + 1.0
    dinv = 1.0 / np.sqrt(deg)

    # self-loops are ordinary messages; table rows are pre-scaled by dinv[src]
    # so every message weight is dinv[dst], folded into the evacuation.
    loop = np.arange(N_NODES, dtype=np.int64)
    ms = np.concatenate([src, loop])
    mt = np.concatenate([dst, loop])

    ms_pos = pos[ms]
    mt_pos = pos[mt]
    core = mt_pos // M_PAD
    lb = (mt_pos % M_PAD) // 128
    dl = mt_pos % 128
    # half table h holds rows [h*MH,(h+1)*MH) of every core, laid out
    # [core, local] so a split AllGather per half fills it contiguously.
    src_local = ms_pos % M_PAD
    half = src_local // MH
    idxl = ((ms_pos // M_PAD) * MH + (src_local % MH)).astype(np.int64)

    # tile ordering: (superblock, half, block-in-sb, k) so one gather call
    # covers all tiles of a (superblock, half) contiguously.
    sb = lb // SB
    bi = lb % SB
    key = ((core * NSB + sb) * 2 + half) * SB + bi
    order = np.argsort(key, kind='stable')
    key_s = key[order]
    idxl_s = idxl[order]
    dl_s = dl[order]

    nkeys = N_CORES * NSB * 2 * SB
    kcounts = np.bincount(key_s, minlength=nkeys).reshape(N_CORES, NSB, 2, SB)
    k_req = -(-kcounts // 128)  # ceil
    K_FIX = k_req.max(axis=0)   # [NSB, 2, SB] uniform across cores
    flat_k = K_FIX.reshape(-1)
    toff = np.zeros(NSB * 2 * SB, np.int64)
    toff[1:] = np.cumsum(flat_k)[:-1]
    toff = toff.reshape(NSB, 2, SB)
    T_TOTAL = int(flat_k.sum())
    # per (sb, half) gather-call tile ranges
    call_t0 = toff[:, :, 0]                      # [NSB, 2]
    call_k = K_FIX.sum(axis=2)                   # [NSB, 2]
    KSBMAX = int(call_k.sum(axis=1).max())       # tiles per sb (both halves)

    # rank of each message within its (core, sb, half, bi) group
    kstart = np.zeros(nkeys, np.int64)
    kstart[1:] = np.cumsum(np.bincount(key_s, minlength=nkeys))[:-1]
    rank = np.arange(len(key_s)) - kstart[key_s]

    core_s = key_s // (NSB * 2 * SB)
    rem = key_s % (NSB * 2 * SB)
    sb_s = rem // (2 * SB)
    half_s = (rem // SB) % 2
    bi_s = rem % SB
    t_glob = toff[sb_s, half_s, bi_s] + rank // 128   # global tile id
    p_slot = rank % 128

    # dl table for on-chip one-hot S build: [core, 128(p), T_TOTAL] fp16.
    # Pad slots get dl = -1 so is_equal(iota, dl) never fires for them.
    dl_all = np.full((N_CORES, 128, T_TOTAL), -1.0, np.float16)
    dl_all[core_s, p_slot, t_glob] = dl_s.astype(np.float16)

    # dma_gather flat order within a (sb, half) call: message i of the call
    # sits at tile call_t0 + i//128, partition i%128, and reads
    # idx[i % 16, call_t0*8 + i // 16].
    i_call = (t_glob - call_t0[sb_s, half_s]) * 128 + p_slot
    colbase = call_t0[sb_s, half_s] * 8
    idx_all = np.zeros((N_CORES, 16, T_TOTAL * 8), np.int16)
    idx_all[core_s, i_call % 16, colbase + i_call // 16] = idxl_s.astype(np.int16)
    idx_rep = np.tile(idx_all, (1, 8, 1))  # [N_CORES, 128, T*8]

    # bias row per core [1, M_PAD] (fp16): sqrt(deg) for real nodes, 0 pads,
    # so (imask x brow) pre-divides the bias by dinv[dst]; the dinv[dst]
    # scale on evacuation restores it. dinvb [core, 128, NB] f32 gives the
    # per-partition dinv[dst] (and dinv[node] for the feature pre-scale).
    imaskf = np.zeros(TOT, np.float32)
    imaskf[pos] = np.sqrt(deg)
    imask = imaskf.reshape(N_CORES, 1, M_PAD).astype(np.float16)
    d_flat = np.zeros(TOT, np.float32)
    d_flat[pos] = dinv.astype(np.float32)
    dinvb = np.ascontiguousarray(
        d_flat.reshape(N_CORES, NB, 128).transpose(0, 2, 1))

    # xT padded per core [D, M_PAD] fp16
    x = np.asarray(x, dtype=np.float32)
    xT_pad = np.zeros((D, TOT), np.float16)
    xT_pad[:, pos] = x.T.astype(np.float16)
    xT_own = np.stack([xT_pad[:, c * M_PAD:(c + 1) * M_PAD] for c in range(N_CORES)])

    KMAXB = int((K_FIX[:, 0, :] + K_FIX[:, 1, :]).max())
    meta = dict(L_PAD=L_PAD, M_PAD=M_PAD, TOT=TOT, HALF=HALF, NB=NB, NSB=NSB,
                K_FIX=K_FIX, toff=toff, T_TOTAL=T_TOTAL,
                call_t0=call_t0, call_k=call_k, KSBMAX=KSBMAX, KMAXB=KMAXB,
                perm_out=perm_out)
    return meta, dl_all, dinvb, idx_rep, imask, xT_own


def _build_bass(meta, weights, repeat=1, ablate=()):
    from concourse import mybir, bacc
    import concourse.tile as tile
    from concourse.masks import make_identity

    L_PAD = meta['L_PAD']; M_PAD = meta['M_PAD']; TOT = meta['TOT']
    HALF = meta['HALF']; NB = meta['NB']; NSB = meta['NSB']
    K_FIX = meta['K_FIX']; toff = meta['toff']; T_TOTAL = meta['T_TOTAL']
    call_t0 = meta['call_t0']; call_k = meta['call_k']; KSBMAX = meta['KSBMAX']
    KMAXB = meta['KMAXB']
    f32 = mybir.dt.float32
    f32r = mybir.dt.float32r
    f16 = mybir.dt.float16
    f8 = mybir.dt.float8e4
    i16 = mybir.dt.int16
    FCH = [(0, 128), (128, 128), (256, 64)]  # feature chunks of 320

    nc = bacc.Bacc("TRN2", target_bir_lowering=False, debug=False,
                   num_devices=N_CORES, num_swdge_queues=4)

    # ---- IO ----
    dl_t = nc.dram_tensor("dl_all", [128, T_TOTAL], f16, kind="ExternalInput")
    idx_t = nc.dram_tensor("idx_all", [128, T_TOTAL * 8], i16, kind="ExternalInput")
    mask_t = nc.dram_tensor("imask", [1, M_PAD], f16, kind="ExternalInput")
    dinvb_t = nc.dram_tensor("dinvb", [128, NB], f32, kind="ExternalInput")
    iota_t = nc.dram_tensor("iota", [128, 128], f16, kind="ExternalInput")
    pmat_t = nc.dram_tensor("Pmat", [N_GRAPHS, N_GRAPHS], f32, kind="ExternalInput")
    xT_t = nc.dram_tensor("xT", [D, M_PAD], f16, kind="ExternalInput")
    W_t = [nc.dram_tensor(f"W{k}", [D, D], f16, kind="ExternalInput") for k in (1, 2, 3)]
    brow_t = [nc.dram_tensor(f"brow{k}", [1, D], f16, kind="ExternalInput") for k in (1, 2, 3)]
    Wf1_t = nc.dram_tensor("Wf1", [320, 256], f32, kind="ExternalInput")
    bf1_t = nc.dram_tensor("bf1c", [128, 2], f32, kind="ExternalInput")
    Wf2_t = nc.dram_tensor("Wf2", [256, 16], f32, kind="ExternalInput")
    bf2_t = nc.dram_tensor("bf2c", [16, 1], f32, kind="ExternalInput")
    Wf3_t = nc.dram_tensor("Wf3", [16, 1], f32, kind="ExternalInput")
    out_t = nc.dram_tensor("out", [1, N_GRAPHS], f32, kind="ExternalOutput")
    bf3_val = float(np.asarray(weights['bf3']).reshape(-1)[0])

    p_own = [nc.dram_tensor(f"p_own{k}", [M_PAD, DP], f8, kind="Internal")
             for k in range(3)]
    p_half = [[nc.dram_tensor(f"p_half{k}_{h}", [HALF, DP], f8, kind="Internal",
                              addr_space="Shared") for h in range(2)]
              for k in range(3)]
    MH = M_PAD // 2
    pooled_own = nc.dram_tensor("pooled_own", [D, GPC], f32, kind="Internal")
    pooled_all = nc.dram_tensor("pooled_all", [N_CORES, D, GPC], f32,
                                kind="Internal", addr_space="Shared")

    RG = [list(range(N_CORES))]

    with tile.TileContext(nc) as tc:
        with tc.tile_pool(name="persist", bufs=1) as pp, \
             tc.tile_pool(name="gpool", bufs=2) as gp, \
             tc.tile_pool(name="spool", bufs=4) as sp, \
             tc.tile_pool(name="hpool", bufs=2) as hp, \
             tc.tile_pool(name="psum_a", bufs=2, space="PSUM") as pa, \
             tc.tile_pool(name="psum_t", bufs=2, space="PSUM") as pt, \
             tc.tile_pool(name="psum_f", bufs=2, space="PSUM") as pf:

            ident_f = pp.tile([128, 128], f32, tag="ident_f")
            make_identity(nc, ident_f[:])
            ident = pp.tile([128, 128], f16, tag="ident")
            nc.vector.tensor_copy(ident[:], ident_f[:])
            idx_sb = pp.tile([128, T_TOTAL * 8], i16, tag="idx")
            nc.sync.dma_start(idx_sb[:], idx_t[:, :])
            dl_sb = pp.tile([128, T_TOTAL], f16, tag="dl")
            nc.sync.dma_start(dl_sb[:], dl_t[:, :])
            iota_sb = pp.tile([128, 128], f16, tag="iota")
            nc.sync.dma_start(iota_sb[:], iota_t[:, :])
            mask_sb = pp.tile([1, M_PAD], f16, tag="mask")
            nc.sync.dma_start(mask_sb[:], mask_t[:, :])
            dinvb_sb = pp.tile([128, NB], f32, tag="dinvb")
            nc.sync.dma_start(dinvb_sb[:], dinvb_t[:, :])
            brow_sb = pp.tile([1, 3 * D], f16, tag="brow")
            for k in range(3):
                nc.sync.dma_start(brow_sb[:, k * D:(k + 1) * D], brow_t[k][:, :])

            # persistent transposed features hT (3 chunks)
            hT = [pp.tile([cl, M_PAD], f16, tag=f"hT{ci}", name=f"hT{ci}")
                  for ci, (cs, cl) in enumerate(FCH)]
            for ci, (cs, cl) in enumerate(FCH):
                nc.sync.dma_start(hT[ci][:], xT_t[cs:cs + cl, :])

            WB = 4  # feature blocks per p_own write

            def load_w(layer):
                wl = []
                for ci, (cs, cl) in enumerate(FCH):
                    wt = hp.tile([cl, D], f16, tag=f"wch{ci}", name=f"wch{ci}_{layer}",
                                 bufs=2)
                    nc.sync.dma_start(wt[:], W_t[layer][cs:cs + cl, :])
                    wl.append(wt)
                return wl

            def feature_group(layer, nb0, wl):
                """WB feature blocks -> one batched p_own write."""
                wb = min(WB, NB - nb0)
                pev = hp.tile([128, WB, D], f8, tag="pev")
                for j in range(wb):
                    nb = nb0 + j
                    ps = pf.tile([128, D], f32, tag="pfeat")
                    for ci, (cs, cl) in enumerate(FCH):
                        nc.tensor.matmul(
                            ps[:],
                            lhsT=hT[ci][:, nb * 128:(nb + 1) * 128],
                            rhs=wl[ci][:],
                            start=(ci == 0), stop=(ci == 2))
                    nc.vector.tensor_scalar(pev[:, j, :], ps[:],
                                            dinvb_sb[:, nb:nb + 1], None,
                                            op0=mybir.AluOpType.mult)
                dst = p_own[layer][nb0 * 128:(nb0 + wb) * 128, 0:D].rearrange(
                    "(b p) c -> p b c", b=wb)
                nc.sync.dma_start(dst, pev[:, 0:wb, :])

            def fire_coll(layer, h):
                if 'collective' in ablate:
                    return
                nc.gpsimd.collective_compute(
                    "AllGather", mybir.AluOpType.bypass, replica_groups=RG,
                    ins=[p_own[layer][h * MH:(h + 1) * MH, :]],
                    outs=[p_half[layer][h][:, :]])

            def feature_phase(layer):
                wl = load_w(layer)
                for nb0 in range(0, NB, WB):
                    feature_group(layer, nb0, wl)
                    if nb0 + WB == NB // 2:
                        fire_coll(layer, 0)
                fire_coll(layer, 1)

            qn = [0]

            def agg_phase(layer, nxt_wl=None):
                """hT = relu(Ahat @ p + b) transposed, per dst block; when
                nxt_wl is given, the next layer's feature blocks and split
                collectives are interleaved so their transfer overlaps."""
                for s in range(NSB):
                    g = gp.tile([128, KSBMAX, DP], f8, tag="g")
                    koff0 = 0
                    for hh in range(2):
                        kh = int(call_k[s, hh])
                        if kh == 0:
                            continue
                        th = int(call_t0[s, hh])
                        gk = 1 if 'gather' in ablate else kh
                        nc.gpsimd.dma_gather(
                            out_ap=g[:, koff0:koff0 + gk, :],
                            in_ap=p_half[layer][hh][:, :],
                            idxs_ap=idx_sb[:, th * 8:(th + gk) * 8],
                            num_idxs=gk * 128,
                            num_idxs_reg=gk * 128,
                            elem_size=DP,
                            single_packet=False,
                            queue_num=qn[0] % 4)
                        qn[0] += 1
                        koff0 += kh
                    tbase = int(call_t0[s, 0])
                    for bi in range(SB):
                        b = s * SB + bi
                        if b >= NB:
                            break
                        ps = pa.tile([128, D], f32, tag="pagg")
                        nc.tensor.matmul(
                            ps[:],
                            lhsT=mask_sb[:, b * 128:(b + 1) * 128],
                            rhs=brow_sb[:, layer * D:(layer + 1) * D],
                            start=True, stop=False)
                        tiles = []
                        sblk = sp.tile([128, KMAXB, 128], f16, tag="sblk")
                        soff = 0
                        for hh in range(2):
                            t0 = int(toff[s, hh, bi])
                            kh = int(K_FIX[s, hh, bi])
                            if kh == 0:
                                continue
                            nc.vector.tensor_tensor(
                                out=sblk[:, soff:soff + kh, :],
                                in0=iota_sb[:].unsqueeze(1).broadcast_to(
                                    [128, kh, 128]),
                                in1=dl_sb[:, t0:t0 + kh].unsqueeze(2).broadcast_to(
                                    [128, kh, 128]),
                                op=mybir.AluOpType.is_equal)
                            for k in range(kh):
                                tiles.append(t0 + k)
                            soff += kh
                        for j, t in enumerate(tiles):
                            rt = 0 if 'gather' in ablate else t - tbase
                            if 'aggmm' not in ablate or j == len(tiles) - 1:
                                nc.tensor.matmul(
                                    ps[:],
                                    lhsT=sblk[:, j, :],
                                    rhs=g[:, rt, 0:D],
                                    start=False, stop=(j == len(tiles) - 1))
                        htmp = hp.tile([128, D], f16, tag="htmp")
                        nc.vector.tensor_scalar(htmp[:], ps[:],
                                                dinvb_sb[:, b:b + 1], 0.0,
                                                op0=mybir.AluOpType.mult,
                                                op1=mybir.AluOpType.max)
                        for ci, (cs, cl) in enumerate(FCH):
                            tp = pt.tile([128, 128], f16, tag="tr")
                            nc.tensor.transpose(tp[:cl, :], htmp[:, cs:cs + cl],
                                                identity=ident[:])
                            nc.any.tensor_copy(hT[ci][:, b * 128:(b + 1) * 128],
                                               tp[:cl, :])
                        if nxt_wl is not None:
                            if b % WB == WB - 1:
                                feature_group(layer + 1, b - (WB - 1), nxt_wl)
                            if b + 1 == NB // 2 + WB:
                                fire_coll(layer + 1, 0)
                if nxt_wl is not None:
                    fire_coll(layer + 1, 1)

            # ---- network ---- (repeat unrolled in python: timing harness only)
            for _rep in range(repeat):
                feature_phase(0)                 # p1 = x @ W1 (+ split AllGather)
                agg_phase(0, nxt_wl=load_w(1))   # h1, p2 + AllGather interleaved
                agg_phase(1, nxt_wl=load_w(2))   # h2, p3 + AllGather interleaved
                agg_phase(2)                     # h3 (lives in hT)

            # ---- global max pool ----
            for ci, (cs, cl) in enumerate(FCH):
                gt = hp.tile([cl, GPC], f32, tag=f"gt{ci}", name=f"gt{ci}", bufs=1)
                for j in range(GPC):
                    nc.vector.reduce_max(
                        gt[:, j:j + 1], hT[ci][:, j * L_PAD:(j + 1) * L_PAD],
                        axis=mybir.AxisListType.X)
                nc.sync.dma_start(pooled_own[cs:cs + cl, :], gt[:])
            nc.gpsimd.collective_compute(
                "AllGather", mybir.AluOpType.bypass, replica_groups=RG,
                ins=[pooled_own[:, :]], outs=[pooled_all[:, :, :]])

            # gT_full chunks [cl, 64]
            gT = []
            for ci, (cs, cl) in enumerate(FCH):
                gtile = hp.tile([cl, N_GRAPHS], f32, tag=f"gTf{ci}", name=f"gTf{ci}", bufs=1)
                for cc in range(N_CORES):
                    nc.sync.dma_start(gtile[:, cc * GPC:(cc + 1) * GPC],
                                      pooled_all[cc, cs:cs + cl, :])
                gT.append(gtile)

            # ---- MLP head (transposed): z1T[256,64] ----
            wf1 = []
            for mi in range(2):
                for ci, (cs, cl) in enumerate(FCH):
                    t = hp.tile([cl, 128], f32, tag=f"wf1_{mi}_{ci}", name=f"wf1_{mi}_{ci}", bufs=1)
                    nc.sync.dma_start(t[:], Wf1_t[cs:cs + cl, mi * 128:(mi + 1) * 128])
                    wf1.append(t)
            bf1sb = hp.tile([128, 2], f32, tag="bf1", bufs=1)
            nc.sync.dma_start(bf1sb[:], bf1_t[:, :])
            h1T = []
            for mi in range(2):
                ps = pf.tile([128, N_GRAPHS], f32, tag="pfeat")
                for ci in range(3):
                    nc.tensor.matmul(ps[:], lhsT=wf1[mi * 3 + ci][:],
                                     rhs=gT[ci][:],
                                     start=(ci == 0), stop=(ci == 2))
                h = hp.tile([128, N_GRAPHS], f32, tag=f"h1T{mi}", name=f"h1T{mi}", bufs=1)
                nc.vector.tensor_scalar(h[:], ps[:],
                                        bf1sb[:, mi:mi + 1], 0.0,
                                        op0=mybir.AluOpType.add,
                                        op1=mybir.AluOpType.max)
                h1T.append(h)
            # z2T [16, 64]
            wf2 = []
            for mi in range(2):
                t = hp.tile([128, 16], f32, tag=f"wf2_{mi}", name=f"wf2_{mi}", bufs=1)
                nc.sync.dma_start(t[:], Wf2_t[mi * 128:(mi + 1) * 128, :])
                wf2.append(t)
            bf2sb = hp.tile([16, 1], f32, tag="bf2", bufs=1)
            nc.sync.dma_start(bf2sb[:], bf2_t[:, :])
            ps2 = pf.tile([16, N_GRAPHS], f32, tag="pfeat")
            for mi in range(2):
                nc.tensor.matmul(ps2[:], lhsT=wf2[mi][:],
                                 rhs=h1T[mi][:],
                                 start=(mi == 0), stop=(mi == 1))
            h2T = hp.tile([16, N_GRAPHS], f32, tag="h2T", bufs=1)
            nc.vector.tensor_scalar(h2T[:], ps2[:], bf2sb[:, 0:1], 0.0,
                                    op0=mybir.AluOpType.add,
                                    op1=mybir.AluOpType.max)
            # z3 as a column [64, 1], then un-permute graphs via P [64, 64]
            wf3 = hp.tile([16, 1], f32, tag="wf3", bufs=1)
            nc.sync.dma_start(wf3[:], Wf3_t[:, :])
            pm_sb = hp.tile([N_GRAPHS, N_GRAPHS], f32, tag="pm", bufs=1)
            nc.sync.dma_start(pm_sb[:], pmat_t[:, :])
            ps3 = pf.tile([N_GRAPHS, 1], f32, tag="pcol")
            nc.tensor.matmul(ps3[:], lhsT=h2T[:],
                             rhs=wf3[:], start=True, stop=True)
            colb = hp.tile([N_GRAPHS, 1], f32, tag="colb", bufs=1)
            nc.vector.tensor_scalar(colb[:], ps3[:], bf3_val, None,
                                    op0=mybir.AluOpType.add)
            ps4 = pf.tile([1, N_GRAPHS], f32, tag="pfeat")
            nc.tensor.matmul(ps4[:], lhsT=colb[:],
                             rhs=pm_sb[:], start=True, stop=True)
            osb = hp.tile([1, N_GRAPHS], f32, tag="osb", bufs=1)
            nc.any.tensor_copy(osb[:], ps4[:])
            nc.sync.dma_start(out_t[:, :], osb[:])

    nc.compile()
    return nc


def _make_runner(nc, in_maps):
    """Build a reusable jitted SPMD executor for `nc` (axon/PJRT path).

    Returns (run_fn, out_names, out_avals): run_fn() executes once and
    returns the list of per-core result dicts.
    """
    import jax
    import numpy as np
    from jax.experimental.shard_map import shard_map
    from jax.sharding import Mesh, NamedSharding, PartitionSpec
    from concourse import bass2jax, mybir

    bass2jax.install_neuronx_cc_hook()
    n_cores = len(in_maps)
    partition_name = nc.partition_id_tensor.name if nc.partition_id_tensor else None
    in_names, out_names, out_avals, zero_outs = [], [], [], []
    for alloc in nc.m.functions[0].allocations:
        if not isinstance(alloc, mybir.MemoryLocationSet):
            continue
        name = alloc.memorylocations[0].name
        if alloc.kind == "ExternalInput":
            if name != partition_name:
                in_names.append(name)
        elif alloc.kind == "ExternalOutput":
            shape = tuple(alloc.tensor_shape)
            dtype = mybir.dt.np(alloc.dtype)
            out_names.append(name)
            out_avals.append(jax.core.ShapedArray(shape, dtype))
            zero_outs.append(np.zeros(shape, dtype))
    n_params = len(in_names)
    n_outs = len(out_avals)
    all_in_names = list(in_names) + list(out_names)
    if partition_name is not None:
        all_in_names.append(partition_name)
    donate = tuple(range(n_params, n_params + n_outs))

    def _body(*args):
        operands = list(args)
        if partition_name is not None:
            operands.append(bass2jax.partition_id_tensor())
        outs = bass2jax._bass_exec_p.bind(
            *operands,
            out_avals=tuple(out_avals),
            in_names=tuple(all_in_names),
            out_names=tuple(out_names),
            lowering_input_output_aliases=(),
            sim_require_finite=True,
            sim_require_nnan=True,
            nc=nc,
        )
        return tuple(outs)

    devices = jax.devices()[:n_cores]
    mesh = Mesh(np.asarray(devices), ("core",))
    in_specs = (PartitionSpec("core"),) * (n_params + n_outs)
    out_specs = (PartitionSpec("core"),) * len(out_names)
    sharded = jax.jit(
        shard_map(_body, mesh=mesh, in_specs=in_specs, out_specs=out_specs,
                  check_rep=False),
        donate_argnums=donate, keep_unused=True)
    sh = NamedSharding(mesh, PartitionSpec("core"))
    concat_in = [
        jax.device_put(
            np.concatenate([np.asarray(in_maps[c][nm]) for c in range(n_cores)],
                           axis=0), sh)
        for nm in in_names
    ]

    def run_fn():
        zeros = [np.zeros((n_cores * z.shape[0], *z.shape[1:]), z.dtype)
                 for z in zero_outs]
        out_arrs = sharded(*concat_in, *zeros)
        out_arrs = [np.asarray(o) for o in out_arrs]
        return [
            {nm: out_arrs[i].reshape(n_cores, *out_avals[i].shape)[c]
             for i, nm in enumerate(out_names)}
            for c in range(n_cores)
        ]

    return run_fn, out_names, out_avals


def prepare(inputs, repeat=1):
    """Preprocess + build + compile; returns a reusable run_fn."""
    meta, dl_all, dinvb, idx_rep, imask, xT_own = _preprocess(
        inputs['x'], inputs['edge_index'], inputs['batch'])
    nc = _build_bass(meta, inputs, repeat=repeat)
    in_maps = _make_in_maps(inputs, meta, dl_all, dinvb, idx_rep, imask, xT_own)
    run_fn, _, _ = _make_runner(nc, in_maps)
    return run_fn


def _make_in_maps(inputs, meta, dl_all, dinvb, idx_rep, imask, xT_own):
    iota = np.tile(np.arange(128, dtype=np.float16)[None, :], (128, 1))
    pmat = np.zeros((N_GRAPHS, N_GRAPHS), np.float32)
    pmat[np.arange(N_GRAPHS), meta['perm_out']] = 1.0
    in_maps = []
    for c in range(N_CORES):
        m = {
            "dl_all": dl_all[c],
            "idx_all": idx_rep[c],
            "imask": imask[c],
            "dinvb": np.ascontiguousarray(dinvb[c]),
            "iota": iota,
            "Pmat": pmat,
            "xT": np.ascontiguousarray(xT_own[c]),
            "Wf1": np.asarray(inputs['Wf1'], np.float32),
            "bf1c": np.ascontiguousarray(
                np.asarray(inputs['bf1'], np.float32).reshape(2, 128).T),
            "Wf2": np.asarray(inputs['Wf2'], np.float32),
            "bf2c": np.asarray(inputs['bf2'], np.float32).reshape(16, 1),
            "Wf3": np.asarray(inputs['Wf3'], np.float32),
        }
        for k in (1, 2, 3):
            m[f"W{k}"] = np.asarray(inputs[f'W{k}'], np.float16)
            m[f"brow{k}"] = np.asarray(inputs[f'b{k}'], np.float16).reshape(1, D)
        in_maps.append(m)
    return in_maps


def kernel(**inputs):
    meta, dl_all, dinvb, idx_rep, imask, xT_own = _preprocess(
        inputs['x'], inputs['edge_index'], inputs['batch'])
    nc = _build_bass(meta, inputs)
    in_maps = _make_in_maps(inputs, meta, dl_all, dinvb, idx_rep, imask, xT_own)
    from concourse.bass_utils import run_bass_kernel_spmd
    res = run_bass_kernel_spmd(nc, in_maps, core_ids=list(range(N_CORES)),
                               trace=False)
    out = np.asarray(res.results[0]["out"]).reshape(1, N_GRAPHS)
    return out.T.copy()
